# revision 12
# baseline (speedup 1.0000x reference)
"""MoE layer (E=8 experts, top-2) on 8 trn2 NeuronCores.

Strategy: expert-centric balanced sharding. The host routes (fp32
logits, top-2, sigmoid gate weights), splits each expert's global token
list into 8 near-equal chunks (one per core), and gathers the x rows
for each (core, expert) chunk into a bf16 [128, 6, capc] tile. Each
core runs the same program: for each expert, stream w1/w2 (bf16,
contiguous per-partition layout, one DMA descriptor per partition),
mm1 (bf16, fp32 accum, PSUM-bank-safe token slices), GELU+bias on the
scalar engine, then mm2 c-partitioned (stationary = w2 tile, tokens
streaming) so there is no partial-token-tile waste, and write the
transposed expert output y^T contiguously to a staging DRAM tensor.
The host applies the top-2 combine weights and scatters into the final
output (0.06% of the FLOPs). The device kernel is a pure back-to-back
matmul stream: the PE never idles after the ~5us startup and the tail
is one small DMA.
"""

import sys
import types

import numpy as np

# Problem constants (nn_MoELayer_46291157516846)
E, C, F, TOPK = 8, 768, 3072, 2
B, T = 8, 2048
N_TOK = B * T
KC1 = C // 128   # 6 contraction chunks for x @ w1
FT = F // 128    # 24 f-tiles of mm1 / contraction chunks of mm2
W1CH = 12        # w1 streamed per expert in 12 chunks of 256 f-columns
W1W = F // W1CH  # 384
W2CH = 4         # w2 streamed per expert in 4 chunks of 6 k-tiles

_CACHE = {}


def _install_ntff_hook():
    """Register the NTFF profiling hook so run_bass_kernel_spmd(trace=True)
    works in this container (antenv.axon_hooks is not shipped)."""
    if "antenv.axon_hooks" in sys.modules:
        return
    mod = types.ModuleType("antenv.axon_hooks")
    mod._hook = None
    mod.set_axon_ntff_profile_hook = lambda h: setattr(mod, "_hook", h)
    mod.get_axon_ntff_profile_hook = lambda: mod._hook
    sys.modules["antenv.axon_hooks"] = mod
    try:
        import antenv

        antenv.axon_hooks = mod
        from trn_agent_boot.trn_boot import _ntff_profile_via_ctypes

        mod.set_axon_ntff_profile_hook(
            _ntff_profile_via_ctypes("/opt/axon/libaxon_pjrt.so")
        )
    except Exception:
        pass


def _slices(capc):
    """Token slices for the moving operand. A single matmul's PSUM
    output must stay within one 2KiB bank (512 fp32), so slice at 512."""
    if capc <= 512:
        return [(0, capc)]
    return [(0, 512), (512, capc - 512)]


def build_program(capcs, order):
    """Build and compile the single-core SPMD Bass program.

    capcs: per-expert compute capacities (multiples of 16, cover the
    max chunk size across cores). order: expert emission order.
    """
    import concourse.bacc as bacc
    import concourse.mybir as mybir
    from concourse.tile import TileContext

    f32 = mybir.dt.float32
    bf16 = mybir.dt.bfloat16
    Act = mybir.ActivationFunctionType

    capcs = list(capcs)
    assert len(capcs) == E and len(order) == E
    yoff = [0] * E  # per-expert offset into the staged y^T free dim
    o = 0
    for e in order:
        yoff[e] = o
        o += 6 * capcs[e]
    GN6 = 6 * sum(capcs)

    nc = bacc.Bacc("TRN2", target_bir_lowering=False, debug=False, num_devices=8)

    xg_in = nc.dram_tensor("xg", [128, GN6], bf16, kind="ExternalInput")
    # contiguous per-partition weight layouts (one DMA descriptor per
    # partition per chunk): see host_prep for the exact element order
    w1_in = nc.dram_tensor("w1p", [E, 128, KC1 * F], bf16, kind="ExternalInput")
    w2_in = nc.dram_tensor("w2p", [E, 128, FT * C], bf16, kind="ExternalInput")
    b1_in = nc.dram_tensor("b1r", [128, E * FT], f32, kind="ExternalInput")
    yt_d = nc.dram_tensor("yt", [128, GN6], f32, kind="ExternalOutput")

    from contextlib import ExitStack

    with TileContext(nc) as tc, ExitStack() as ctx:
        consts = ctx.enter_context(tc.tile_pool(name="consts", bufs=1))
        ppA = ctx.enter_context(tc.tile_pool(name="ppA", bufs=2, space="PSUM"))
        ppB = ctx.enter_context(tc.tile_pool(name="ppB", bufs=2, space="PSUM"))
        pw1 = ctx.enter_context(tc.tile_pool(name="pw1", bufs=18))
        pw2 = ctx.enter_context(tc.tile_pool(name="pw2", bufs=5))
        pxg = ctx.enter_context(tc.tile_pool(name="pxg", bufs=2))
        ph = ctx.enter_context(tc.tile_pool(name="ph", bufs=2))
        pys = ctx.enter_context(tc.tile_pool(name="pys", bufs=3))

        # ---------- small tables (scalar HWDGE queue, land first) ----------
        b1_sb = consts.tile([128, E, FT], f32)
        nc.scalar.dma_start(
            out=b1_sb, in_=b1_in.ap().rearrange("p (e t) -> p e t", e=E)
        )

        # ---------- PE warm-up during the initial DMA wait ----------
        z128 = consts.tile([128, 128], bf16)
        nc.vector.memset(z128, 0.0)
        warm = ppB.tile([128, 128], f32, tag="py", name="warm")
        for _ in range(32):
            nc.tensor.matmul(warm, z128, z128, start=True, stop=True)

        W1C = KC1 * W1W   # 2304 elements per w1 chunk per partition
        W2C = 6 * C       # 4608 elements per w2 chunk per partition

        def emit_loads(e, first=False):
            # the first expert's xg/w2 ride the scalar HWDGE ring so the
            # critical w1 stream owns the sync ring from t=0
            xq = nc.scalar if first else nc.sync
            capc = capcs[e]
            xgt = pxg.tile([128, 6, capc], bf16, tag="xg", name=f"xg{e}")
            xq.dma_start(
                out=xgt,
                in_=xg_in.ap()[:, yoff[e]:yoff[e] + 6 * capc].rearrange(
                    "p (k t) -> p k t", k=6
                ),
            )
            w1c = []
            for q in range(W1CH):
                t = pw1.tile([128, 6, W1W], bf16, tag="w1", name=f"w1_{e}_{q}")
                nc.sync.dma_start(
                    out=t,
                    in_=w1_in.ap()[e][:, q * W1C:(q + 1) * W1C].rearrange(
                        "p (k f) -> p k f", k=6
                    ),
                )
                w1c.append(t)
            w2c = []
            for q in range(W2CH):
                t = pw2.tile([128, 6, C], bf16, tag="w2", name=f"w2_{e}_{q}")
                xq.dma_start(
                    out=t,
                    in_=w2_in.ap()[e][:, q * W2C:(q + 1) * W2C].rearrange(
                        "p (k c) -> p k c", k=6
                    ),
                )
                w2c.append(t)
            return xgt, w1c, w2c

        FPC = W1W // 128  # f-tiles per w1 chunk

        def emit_mm1(e, xgt, w1c, h):
            capc = capcs[e]
            nsl = _slices(capc)
            for ft in range(FT):
                wt = w1c[ft // FPC]
                fc = (ft % FPC) * 128
                psh = ppA.tile([128, capc], f32, tag="pp", name=f"psh{e}_{ft}")
                for k in range(KC1):
                    lhsT = wt[:, k, fc:fc + 128]
                    for ns, nw in nsl:
                        nc.tensor.matmul(
                            psh[:, ns:ns + nw], lhsT, xgt[:, k, ns:ns + nw],
                            start=(k == 0), stop=(k == KC1 - 1),
                        )
                nc.scalar.activation(
                    h[:, ft, :], psh, Act.Gelu,
                    bias=b1_sb[:, e, ft:ft + 1], scale=1.0,
                )

        def emit_mm2(e, h, w2c):
            """mm2 c-partitioned: out^T[c_tile, tok] += w2_chunk.T @ h_chunk.
            No partial-token-tile waste; y^T goes straight to DRAM."""
            capc = capcs[e]
            nsl = _slices(capc)
            for ct in range(KC1):
                psz = ppB.tile([128, capc], f32, tag="py", name=f"psz{e}_{ct}")
                for k in range(FT):
                    wq = w2c[k // 6]
                    lhsT = wq[:, k % 6, ct * 128:(ct + 1) * 128]
                    for ns, nw in nsl:
                        nc.tensor.matmul(
                            psz[:, ns:ns + nw], lhsT, h[:, k, ns:ns + nw],
                            start=(k == 0), stop=(k == FT - 1),
                        )
                ysb = pys.tile([128, capc], f32, tag="ysb", name=f"ys{e}_{ct}")
                nc.vector.tensor_copy(ysb, psz)
                nc.sync.dma_start(
                    out=yt_d.ap()[
                        :, yoff[e] + ct * capc:yoff[e] + (ct + 1) * capc
                    ],
                    in_=ysb,
                )

        for i, e in enumerate(order):
            xgt, w1c, w2c = emit_loads(e, first=(i == 0))
            h = ph.tile([128, FT, capcs[e]], bf16, tag="h", name=f"h{e}")
            emit_mm1(e, xgt, w1c, h)
            emit_mm2(e, h, w2c)

    nc.compile()
    return nc


def _route(x, router_w):
    """Host routing on the full batch: per-expert global token lists,
    combine weights, balanced per-core chunks."""
    x = np.asarray(x, np.float32).reshape(N_TOK, C)
    rw = np.asarray(router_w, np.float32)
    lg = x @ rw.T                                          # [N, E]
    order2 = np.argsort(-lg, axis=-1, kind="stable")[:, :TOPK]
    m1 = np.take_along_axis(lg, order2[:, 0:1], axis=-1)[:, 0]
    m2 = np.take_along_axis(lg, order2[:, 1:2], axis=-1)[:, 0]
    g1 = 1.0 / (1.0 + np.exp((m2 - m1).astype(np.float64)))
    wts = np.stack([g1, 1.0 - g1], axis=-1).astype(np.float32)  # [N, 2]

    glists, wlists = [], []
    for e in range(E):
        sel = order2 == e                                  # [N, 2]
        any_ = sel.any(-1)
        toks = np.nonzero(any_)[0]
        w = wts[any_][sel[any_]]
        glists.append(toks)
        wlists.append(w.astype(np.float32))
    return glists, wlists


def host_prep(x, router_w, w1, b1, w2, b2, routing=None):
    """Balanced shard + lay out inputs for the 8 cores. Returns
    (in_maps, meta); meta drives the host-side combine in assemble()."""
    from ml_dtypes import bfloat16

    x = np.asarray(x, np.float32).reshape(N_TOK, C)
    router_w = np.asarray(router_w, np.float32)
    w1 = np.asarray(w1, np.float32)
    b1 = np.asarray(b1, np.float32)
    w2 = np.asarray(w2, np.float32)

    if routing is None:
        routing = _route(x, router_w)
    glists, wlists = routing
    chunks = [np.array_split(np.arange(len(glists[e])), B) for e in range(E)]
    capcs = [
        int((max(len(c) for c in chunks[e]) + 7) // 8 * 8) for e in range(E)
    ]
    order = sorted(range(E), key=lambda e: -capcs[e])

    # contiguous per-partition weight layouts:
    # w1p[e, p, q*2304 + k*384 + f'] = w1[e, k*128+p, q*384+f']
    w1b = w1.astype(bfloat16)
    w1p = np.ascontiguousarray(
        w1b.reshape(E, KC1, 128, W1CH, W1W).transpose(0, 2, 3, 1, 4)
        .reshape(E, 128, KC1 * F)
    )
    # w2p[e, p, q*4608 + kk*768 + c] = w2[e, (q*6+kk)*128+p, c]
    w2b = w2.astype(bfloat16)
    w2p = np.ascontiguousarray(
        w2b.reshape(E, W2CH, 6, 128, C).transpose(0, 3, 1, 2, 4)
        .reshape(E, 128, FT * C)
    )
    b1r = np.ascontiguousarray(
        b1.reshape(E, FT, 128).transpose(2, 0, 1).reshape(128, E * FT)
    )
    shared = {"w1p": w1p, "w2p": w2p, "b1r": b1r}

    xb = x.astype(bfloat16)
    GN6 = 6 * sum(capcs)
    maps = []
    for core in range(B):
        xg = np.zeros((128, GN6), bfloat16)
        xo = 0
        for e in order:
            capc = capcs[e]
            idx = chunks[e][core]
            toks = glists[e][idx]
            n = len(toks)
            g = np.zeros((capc, KC1, 128), bfloat16)
            g[:n] = xb[toks].reshape(n, KC1, 128)
            xg[:, xo:xo + 6 * capc] = (
                g.transpose(2, 1, 0).reshape(128, 6 * capc)
            )
            xo += 6 * capc
        maps.append({"xg": xg, **shared})

    meta = {
        "capcs": tuple(capcs),
        "order": tuple(order),
        "glists": glists,
        "wlists": wlists,
        "chunks": chunks,
    }
    return maps, meta


def assemble(res, meta, b2):
    """Host combine: out[tok] += w_e * y_e^T (transposed back), plus the
    exact b2 contribution (sum of the token's two gate weights is 1)."""
    capcs, order = meta["capcs"], meta["order"]
    glists, wlists, chunks = meta["glists"], meta["wlists"], meta["chunks"]
    out = np.zeros((N_TOK, C), np.float32)
    for core in range(B):
        yt = res.results[core]["yt"]
        yo = 0
        for e in order:
            capc = capcs[e]
            idx = chunks[e][core]
            n = len(idx)
            if n:
                y = (
                    yt[:, yo:yo + 6 * capc]
                    .reshape(128, 6, capc)[:, :, :n]
                    .transpose(2, 1, 0)
                    .reshape(n, C)
                )
                toks = glists[e][idx]
                out[toks] += wlists[e][idx][:, None] * y
            yo += 6 * capc
    b2 = np.asarray(b2, np.float32)
    if np.any(b2):
        for e in range(E):
            out[glists[e]] += wlists[e][:, None] * b2[e]
    return out.reshape(B, T, C)


def kernel(**inputs):
    _install_ntff_hook()
    from concourse import bass_utils

    in_maps, meta = host_prep(
        inputs["x"], inputs["router_w"], inputs["w1"],
        inputs["b1"], inputs["w2"], inputs["b2"],
    )
    key = ("nc", meta["capcs"], meta["order"])
    if key not in _CACHE:
        _CACHE[key] = build_program(capcs=meta["capcs"], order=meta["order"])
    nc = _CACHE[key]

    res = bass_utils.run_bass_kernel_spmd(
        nc, in_maps, core_ids=list(range(B)), trace=False
    )
    _CACHE["nc"] = nc
    _CACHE["meta"] = meta
    return assemble(res, meta, inputs["b2"])


# revision 13
# speedup vs baseline: 1.0139x; 1.0139x over previous
"""MoE layer (E=8 experts, top-2) on 8 trn2 NeuronCores.

Strategy: expert-centric balanced sharding. The host routes (fp32
logits, top-2, sigmoid gate weights), splits each expert's global token
list into 8 near-equal chunks (one per core), and gathers the x rows
for each (core, expert) chunk into a bf16 [128, 6, capc] tile. Each
core runs the same program: for each expert, stream w1/w2 (bf16,
contiguous per-partition layout, one DMA descriptor per partition),
mm1 (bf16, fp32 accum, PSUM-bank-safe token slices), GELU+bias on the
scalar engine, then mm2 c-partitioned (stationary = w2 tile, tokens
streaming) so there is no partial-token-tile waste, and write the
transposed expert output y^T contiguously to a staging DRAM tensor.
The host applies the top-2 combine weights and scatters into the final
output (0.06% of the FLOPs). The device kernel is a pure back-to-back
matmul stream: the PE never idles after the ~5us startup and the tail
is one small DMA.
"""

import sys
import types

import numpy as np

# Problem constants (nn_MoELayer_46291157516846)
E, C, F, TOPK = 8, 768, 3072, 2
B, T = 8, 2048
N_TOK = B * T
KC1 = C // 128   # 6 contraction chunks for x @ w1
FT = F // 128    # 24 f-tiles of mm1 / contraction chunks of mm2
W1CH = 12        # w1 streamed per expert in 12 chunks of 256 f-columns
W1W = F // W1CH  # 384
W2CH = 4         # w2 streamed per expert in 4 chunks of 6 k-tiles

_CACHE = {}


def _install_ntff_hook():
    """Register the NTFF profiling hook so run_bass_kernel_spmd(trace=True)
    works in this container (antenv.axon_hooks is not shipped)."""
    if "antenv.axon_hooks" in sys.modules:
        return
    mod = types.ModuleType("antenv.axon_hooks")
    mod._hook = None
    mod.set_axon_ntff_profile_hook = lambda h: setattr(mod, "_hook", h)
    mod.get_axon_ntff_profile_hook = lambda: mod._hook
    sys.modules["antenv.axon_hooks"] = mod
    try:
        import antenv

        antenv.axon_hooks = mod
        from trn_agent_boot.trn_boot import _ntff_profile_via_ctypes

        mod.set_axon_ntff_profile_hook(
            _ntff_profile_via_ctypes("/opt/axon/libaxon_pjrt.so")
        )
    except Exception:
        pass


def _slices(capc):
    """Token slices for the moving operand. A single matmul's PSUM
    output must stay within one 2KiB bank (512 fp32), so slice at 512."""
    if capc <= 512:
        return [(0, capc)]
    return [(0, 512), (512, capc - 512)]


def build_program(capcs, order):
    """Build and compile the single-core SPMD Bass program.

    capcs: per-expert compute capacities (multiples of 16, cover the
    max chunk size across cores). order: expert emission order.
    """
    import concourse.bacc as bacc
    import concourse.mybir as mybir
    from concourse.tile import TileContext

    f32 = mybir.dt.float32
    bf16 = mybir.dt.bfloat16
    Act = mybir.ActivationFunctionType

    capcs = list(capcs)
    assert len(capcs) == E and len(order) == E
    yoff = [0] * E  # per-expert offset into the staged y^T free dim
    o = 0
    for e in order:
        yoff[e] = o
        o += 6 * capcs[e]
    GN6 = 6 * sum(capcs)

    nc = bacc.Bacc("TRN2", target_bir_lowering=False, debug=False, num_devices=8)

    xg_in = nc.dram_tensor("xg", [128, GN6], bf16, kind="ExternalInput")
    # contiguous per-partition weight layouts (one DMA descriptor per
    # partition per chunk): see host_prep for the exact element order
    w1_in = nc.dram_tensor("w1p", [E, 128, KC1 * F], bf16, kind="ExternalInput")
    w2_in = nc.dram_tensor("w2p", [E, 128, FT * C], bf16, kind="ExternalInput")
    b1_in = nc.dram_tensor("b1r", [128, E * FT], f32, kind="ExternalInput")
    yt_d = nc.dram_tensor("yt", [128, GN6], f32, kind="ExternalOutput")

    from contextlib import ExitStack

    with TileContext(nc) as tc, ExitStack() as ctx:
        consts = ctx.enter_context(tc.tile_pool(name="consts", bufs=1))
        ppA = ctx.enter_context(tc.tile_pool(name="ppA", bufs=2, space="PSUM"))
        ppB = ctx.enter_context(tc.tile_pool(name="ppB", bufs=2, space="PSUM"))
        pw1 = ctx.enter_context(tc.tile_pool(name="pw1", bufs=18))
        pw2 = ctx.enter_context(tc.tile_pool(name="pw2", bufs=5))
        pxg = ctx.enter_context(tc.tile_pool(name="pxg", bufs=2))
        ph = ctx.enter_context(tc.tile_pool(name="ph", bufs=2))
        pys = ctx.enter_context(tc.tile_pool(name="pys", bufs=3))

        # ---------- small tables (scalar HWDGE queue, land first) ----------
        b1_sb = consts.tile([128, E, FT], f32)
        nc.scalar.dma_start(
            out=b1_sb, in_=b1_in.ap().rearrange("p (e t) -> p e t", e=E)
        )

        # ---------- PE warm-up during the initial DMA wait ----------
        z128 = consts.tile([128, 128], bf16)
        nc.vector.memset(z128, 0.0)
        warm = ppB.tile([128, 128], f32, tag="py", name="warm")
        for _ in range(32):
            nc.tensor.matmul(warm, z128, z128, start=True, stop=True)

        W1C = KC1 * W1W   # 2304 elements per w1 chunk per partition
        W2C = 6 * C       # 4608 elements per w2 chunk per partition

        def emit_loads(e, first=False):
            # startup: the first expert's xg and first w1 chunk ride the
            # scalar HWDGE ring so mm1 can start while the sync ring
            # streams the remaining w1 chunks; everything else stays on
            # the sync ring in consumption order (w1 then w2) so the
            # critical stream owns the HBM bandwidth
            xq = nc.scalar if first else nc.sync
            capc = capcs[e]
            xgt = pxg.tile([128, 6, capc], bf16, tag="xg", name=f"xg{e}")
            xq.dma_start(
                out=xgt,
                in_=xg_in.ap()[:, yoff[e]:yoff[e] + 6 * capc].rearrange(
                    "p (k t) -> p k t", k=6
                ),
            )
            w1c = []
            for q in range(W1CH):
                t = pw1.tile([128, 6, W1W], bf16, tag="w1", name=f"w1_{e}_{q}")
                eng = nc.scalar if (first and q == 0) else nc.sync
                eng.dma_start(
                    out=t,
                    in_=w1_in.ap()[e][:, q * W1C:(q + 1) * W1C].rearrange(
                        "p (k f) -> p k f", k=6
                    ),
                )
                w1c.append(t)
            w2c = []
            for q in range(W2CH):
                t = pw2.tile([128, 6, C], bf16, tag="w2", name=f"w2_{e}_{q}")
                nc.sync.dma_start(
                    out=t,
                    in_=w2_in.ap()[e][:, q * W2C:(q + 1) * W2C].rearrange(
                        "p (k c) -> p k c", k=6
                    ),
                )
                w2c.append(t)
            return xgt, w1c, w2c

        FPC = W1W // 128  # f-tiles per w1 chunk

        def emit_mm1(e, xgt, w1c, h):
            capc = capcs[e]
            nsl = _slices(capc)
            for ft in range(FT):
                wt = w1c[ft // FPC]
                fc = (ft % FPC) * 128
                psh = ppA.tile([128, capc], f32, tag="pp", name=f"psh{e}_{ft}")
                for k in range(KC1):
                    lhsT = wt[:, k, fc:fc + 128]
                    for ns, nw in nsl:
                        nc.tensor.matmul(
                            psh[:, ns:ns + nw], lhsT, xgt[:, k, ns:ns + nw],
                            start=(k == 0), stop=(k == KC1 - 1),
                        )
                nc.scalar.activation(
                    h[:, ft, :], psh, Act.Gelu,
                    bias=b1_sb[:, e, ft:ft + 1], scale=1.0,
                )

        def emit_mm2(e, h, w2c):
            """mm2 c-partitioned: out^T[c_tile, tok] += w2_chunk.T @ h_chunk.
            No partial-token-tile waste; y^T goes straight to DRAM."""
            capc = capcs[e]
            nsl = _slices(capc)
            for ct in range(KC1):
                psz = ppB.tile([128, capc], f32, tag="py", name=f"psz{e}_{ct}")
                for k in range(FT):
                    wq = w2c[k // 6]
                    lhsT = wq[:, k % 6, ct * 128:(ct + 1) * 128]
                    for ns, nw in nsl:
                        nc.tensor.matmul(
                            psz[:, ns:ns + nw], lhsT, h[:, k, ns:ns + nw],
                            start=(k == 0), stop=(k == FT - 1),
                        )
                ysb = pys.tile([128, capc], f32, tag="ysb", name=f"ys{e}_{ct}")
                nc.vector.tensor_copy(ysb, psz)
                nc.sync.dma_start(
                    out=yt_d.ap()[
                        :, yoff[e] + ct * capc:yoff[e] + (ct + 1) * capc
                    ],
                    in_=ysb,
                )

        for i, e in enumerate(order):
            xgt, w1c, w2c = emit_loads(e, first=(i == 0))
            h = ph.tile([128, FT, capcs[e]], bf16, tag="h", name=f"h{e}")
            emit_mm1(e, xgt, w1c, h)
            emit_mm2(e, h, w2c)

    nc.compile()
    return nc


def _route(x, router_w):
    """Host routing on the full batch: per-expert global token lists,
    combine weights, balanced per-core chunks."""
    x = np.asarray(x, np.float32).reshape(N_TOK, C)
    rw = np.asarray(router_w, np.float32)
    lg = x @ rw.T                                          # [N, E]
    order2 = np.argsort(-lg, axis=-1, kind="stable")[:, :TOPK]
    m1 = np.take_along_axis(lg, order2[:, 0:1], axis=-1)[:, 0]
    m2 = np.take_along_axis(lg, order2[:, 1:2], axis=-1)[:, 0]
    g1 = 1.0 / (1.0 + np.exp((m2 - m1).astype(np.float64)))
    wts = np.stack([g1, 1.0 - g1], axis=-1).astype(np.float32)  # [N, 2]

    glists, wlists = [], []
    for e in range(E):
        sel = order2 == e                                  # [N, 2]
        any_ = sel.any(-1)
        toks = np.nonzero(any_)[0]
        w = wts[any_][sel[any_]]
        glists.append(toks)
        wlists.append(w.astype(np.float32))
    return glists, wlists


def host_prep(x, router_w, w1, b1, w2, b2, routing=None):
    """Balanced shard + lay out inputs for the 8 cores. Returns
    (in_maps, meta); meta drives the host-side combine in assemble()."""
    from ml_dtypes import bfloat16

    x = np.asarray(x, np.float32).reshape(N_TOK, C)
    router_w = np.asarray(router_w, np.float32)
    w1 = np.asarray(w1, np.float32)
    b1 = np.asarray(b1, np.float32)
    w2 = np.asarray(w2, np.float32)

    if routing is None:
        routing = _route(x, router_w)
    glists, wlists = routing
    chunks = [np.array_split(np.arange(len(glists[e])), B) for e in range(E)]
    capcs = [
        int((max(len(c) for c in chunks[e]) + 7) // 8 * 8) for e in range(E)
    ]
    order = sorted(range(E), key=lambda e: -capcs[e])

    # contiguous per-partition weight layouts:
    # w1p[e, p, q*2304 + k*384 + f'] = w1[e, k*128+p, q*384+f']
    w1b = w1.astype(bfloat16)
    w1p = np.ascontiguousarray(
        w1b.reshape(E, KC1, 128, W1CH, W1W).transpose(0, 2, 3, 1, 4)
        .reshape(E, 128, KC1 * F)
    )
    # w2p[e, p, q*4608 + kk*768 + c] = w2[e, (q*6+kk)*128+p, c]
    w2b = w2.astype(bfloat16)
    w2p = np.ascontiguousarray(
        w2b.reshape(E, W2CH, 6, 128, C).transpose(0, 3, 1, 2, 4)
        .reshape(E, 128, FT * C)
    )
    b1r = np.ascontiguousarray(
        b1.reshape(E, FT, 128).transpose(2, 0, 1).reshape(128, E * FT)
    )
    shared = {"w1p": w1p, "w2p": w2p, "b1r": b1r}

    xb = x.astype(bfloat16)
    GN6 = 6 * sum(capcs)
    maps = []
    for core in range(B):
        xg = np.zeros((128, GN6), bfloat16)
        xo = 0
        for e in order:
            capc = capcs[e]
            idx = chunks[e][core]
            toks = glists[e][idx]
            n = len(toks)
            g = np.zeros((capc, KC1, 128), bfloat16)
            g[:n] = xb[toks].reshape(n, KC1, 128)
            xg[:, xo:xo + 6 * capc] = (
                g.transpose(2, 1, 0).reshape(128, 6 * capc)
            )
            xo += 6 * capc
        maps.append({"xg": xg, **shared})

    meta = {
        "capcs": tuple(capcs),
        "order": tuple(order),
        "glists": glists,
        "wlists": wlists,
        "chunks": chunks,
    }
    return maps, meta


def assemble(res, meta, b2):
    """Host combine: out[tok] += w_e * y_e^T (transposed back), plus the
    exact b2 contribution (sum of the token's two gate weights is 1)."""
    capcs, order = meta["capcs"], meta["order"]
    glists, wlists, chunks = meta["glists"], meta["wlists"], meta["chunks"]
    out = np.zeros((N_TOK, C), np.float32)
    for core in range(B):
        yt = res.results[core]["yt"]
        yo = 0
        for e in order:
            capc = capcs[e]
            idx = chunks[e][core]
            n = len(idx)
            if n:
                y = (
                    yt[:, yo:yo + 6 * capc]
                    .reshape(128, 6, capc)[:, :, :n]
                    .transpose(2, 1, 0)
                    .reshape(n, C)
                )
                toks = glists[e][idx]
                out[toks] += wlists[e][idx][:, None] * y
            yo += 6 * capc
    b2 = np.asarray(b2, np.float32)
    if np.any(b2):
        for e in range(E):
            out[glists[e]] += wlists[e][:, None] * b2[e]
    return out.reshape(B, T, C)


def kernel(**inputs):
    _install_ntff_hook()
    from concourse import bass_utils

    in_maps, meta = host_prep(
        inputs["x"], inputs["router_w"], inputs["w1"],
        inputs["b1"], inputs["w2"], inputs["b2"],
    )
    key = ("nc", meta["capcs"], meta["order"])
    if key not in _CACHE:
        _CACHE[key] = build_program(capcs=meta["capcs"], order=meta["order"])
    nc = _CACHE[key]

    res = bass_utils.run_bass_kernel_spmd(
        nc, in_maps, core_ids=list(range(B)), trace=False
    )
    _CACHE["nc"] = nc
    _CACHE["meta"] = meta
    return assemble(res, meta, inputs["b2"])


# revision 18
# speedup vs baseline: 1.0197x; 1.0056x over previous
"""MoE layer (E=8 experts, top-2) on 8 trn2 NeuronCores.

Strategy: expert-centric balanced sharding. The host routes (fp32
logits, top-2, sigmoid gate weights), splits each expert's global token
list into 8 near-equal chunks (one per core), and gathers the x rows
for each (core, expert) chunk into a bf16 [128, 6, capc] tile. Each
core runs the same program: for each expert, stream w1/w2 (bf16,
contiguous per-partition layout, one DMA descriptor per partition),
mm1 (bf16, fp32 accum, PSUM-bank-safe token slices), GELU+bias on the
scalar engine, then mm2 c-partitioned (stationary = w2 tile, tokens
streaming) so there is no partial-token-tile waste, and write the
transposed expert output y^T contiguously to a staging DRAM tensor.
The host applies the top-2 combine weights and scatters into the final
output (0.06% of the FLOPs). The device kernel is a pure back-to-back
matmul stream: the PE never idles after the ~5us startup and the tail
is one small DMA.
"""

import sys
import types

import numpy as np

# Problem constants (nn_MoELayer_46291157516846)
E, C, F, TOPK = 8, 768, 3072, 2
B, T = 8, 2048
N_TOK = B * T
KC1 = C // 128   # 6 contraction chunks for x @ w1
FT = F // 128    # 24 f-tiles of mm1 / contraction chunks of mm2
W1CH = 12        # w1 streamed per expert in 12 chunks of 256 f-columns
W1W = F // W1CH  # 384
W2CH = 4         # w2 streamed per expert in 4 chunks of 6 k-tiles

_CACHE = {}


def _install_ntff_hook():
    """Register the NTFF profiling hook so run_bass_kernel_spmd(trace=True)
    works in this container (antenv.axon_hooks is not shipped)."""
    if "antenv.axon_hooks" in sys.modules:
        return
    mod = types.ModuleType("antenv.axon_hooks")
    mod._hook = None
    mod.set_axon_ntff_profile_hook = lambda h: setattr(mod, "_hook", h)
    mod.get_axon_ntff_profile_hook = lambda: mod._hook
    sys.modules["antenv.axon_hooks"] = mod
    try:
        import antenv

        antenv.axon_hooks = mod
        from trn_agent_boot.trn_boot import _ntff_profile_via_ctypes

        mod.set_axon_ntff_profile_hook(
            _ntff_profile_via_ctypes("/opt/axon/libaxon_pjrt.so")
        )
    except Exception:
        pass


def _slices(capc):
    """Token slices for the moving operand. A single matmul's PSUM
    output must stay within one 2KiB bank (512 fp32), so slice at 512."""
    if capc <= 512:
        return [(0, capc)]
    return [(0, 512), (512, capc - 512)]


def build_program(capcs, order):
    """Build and compile the single-core SPMD Bass program.

    capcs: per-expert compute capacities (multiples of 16, cover the
    max chunk size across cores). order: expert emission order.
    """
    import concourse.bacc as bacc
    import concourse.mybir as mybir
    from concourse.tile import TileContext
    from concourse.tile_rust import add_dep_helper

    f32 = mybir.dt.float32
    bf16 = mybir.dt.bfloat16
    Act = mybir.ActivationFunctionType

    capcs = list(capcs)
    assert len(capcs) == E and len(order) == E
    yoff = [0] * E  # per-expert offset into the staged y^T free dim
    o = 0
    for e in order:
        yoff[e] = o
        o += 6 * capcs[e]
    GN6 = 6 * sum(capcs)

    nc = bacc.Bacc("TRN2", target_bir_lowering=False, debug=False, num_devices=8)

    xg_in = nc.dram_tensor("xg", [128, GN6], bf16, kind="ExternalInput")
    # contiguous per-partition weight layouts (one DMA descriptor per
    # partition per chunk): see host_prep for the exact element order
    w1_in = nc.dram_tensor("w1p", [E, 128, KC1 * F], bf16, kind="ExternalInput")
    w2_in = nc.dram_tensor("w2p", [E, 128, FT * C], bf16, kind="ExternalInput")
    b1_in = nc.dram_tensor("b1r", [128, E * FT], f32, kind="ExternalInput")
    yt_d = nc.dram_tensor("yt", [128, GN6], f32, kind="ExternalOutput")

    from contextlib import ExitStack

    with TileContext(nc) as tc, ExitStack() as ctx:
        consts = ctx.enter_context(tc.tile_pool(name="consts", bufs=1))
        ppA = ctx.enter_context(tc.tile_pool(name="ppA", bufs=2, space="PSUM"))
        ppB = ctx.enter_context(tc.tile_pool(name="ppB", bufs=2, space="PSUM"))
        pw1 = ctx.enter_context(tc.tile_pool(name="pw1", bufs=18))
        pw2 = ctx.enter_context(tc.tile_pool(name="pw2", bufs=5))
        pxg = ctx.enter_context(tc.tile_pool(name="pxg", bufs=2))
        ph = ctx.enter_context(tc.tile_pool(name="ph", bufs=2))
        pys = ctx.enter_context(tc.tile_pool(name="pys", bufs=3))

        # ---------- small tables (scalar HWDGE queue, land first) ----------
        b1_sb = consts.tile([128, E, FT], f32)
        nc.scalar.dma_start(
            out=b1_sb, in_=b1_in.ap().rearrange("p (e t) -> p e t", e=E)
        )

        # ---------- PE warm-up during the initial DMA wait ----------
        z128 = consts.tile([128, 128], bf16)
        nc.vector.memset(z128, 0.0)
        warm = ppB.tile([128, 128], f32, tag="py", name="warm")
        for _ in range(32):
            nc.tensor.matmul(warm, z128, z128, start=True, stop=True)

        W1C = KC1 * W1W   # 2304 elements per w1 chunk per partition
        W2C = 6 * C       # 4608 elements per w2 chunk per partition

        # the input ring (sync queue) is strictly chained in consumption
        # order — the tile scheduler otherwise reorders the queue and a
        # late-needed w2 block can starve the critical w1 stream
        chain = [None]

        def ring_dma(out, in_):
            ins = nc.sync.dma_start(out=out, in_=in_)
            if chain[0] is not None:
                add_dep_helper(
                    ins.ins, chain[0].ins, sync=False,
                    reason="input ring consumption order",
                )
            chain[0] = ins
            return ins

        def emit_loads(e):
            capc = capcs[e]
            xgt = pxg.tile([128, 6, capc], bf16, tag="xg", name=f"xg{e}")
            ring_dma(
                xgt,
                xg_in.ap()[:, yoff[e]:yoff[e] + 6 * capc].rearrange(
                    "p (k t) -> p k t", k=6
                ),
            )
            w1c = []
            for q in range(W1CH):
                t = pw1.tile([128, 6, W1W], bf16, tag="w1", name=f"w1_{e}_{q}")
                ring_dma(
                    t,
                    w1_in.ap()[e][:, q * W1C:(q + 1) * W1C].rearrange(
                        "p (k f) -> p k f", k=6
                    ),
                )
                w1c.append(t)
            w2c = []
            for q in range(W2CH):
                t = pw2.tile([128, 6, C], bf16, tag="w2", name=f"w2_{e}_{q}")
                ring_dma(
                    t,
                    w2_in.ap()[e][:, q * W2C:(q + 1) * W2C].rearrange(
                        "p (k c) -> p k c", k=6
                    ),
                )
                w2c.append(t)
            return xgt, w1c, w2c

        FPC = W1W // 128  # f-tiles per w1 chunk

        def emit_mm1(e, xgt, w1c, h):
            capc = capcs[e]
            nsl = _slices(capc)
            for ft in range(FT):
                wt = w1c[ft // FPC]
                fc = (ft % FPC) * 128
                psh = ppA.tile([128, capc], f32, tag="pp", name=f"psh{e}_{ft}")
                for k in range(KC1):
                    lhsT = wt[:, k, fc:fc + 128]
                    for ns, nw in nsl:
                        nc.tensor.matmul(
                            psh[:, ns:ns + nw], lhsT, xgt[:, k, ns:ns + nw],
                            start=(k == 0), stop=(k == KC1 - 1),
                        )
                nc.scalar.activation(
                    h[:, ft, :], psh, Act.Gelu,
                    bias=b1_sb[:, e, ft:ft + 1], scale=1.0,
                )

        def emit_mm2(e, h, w2c):
            """mm2 c-partitioned: out^T[c_tile, tok] += w2_chunk.T @ h_chunk.
            No partial-token-tile waste; y^T goes straight to DRAM."""
            capc = capcs[e]
            nsl = _slices(capc)
            for ct in range(KC1):
                psz = ppB.tile([128, capc], f32, tag="py", name=f"psz{e}_{ct}")
                for k in range(FT):
                    wq = w2c[k // 6]
                    lhsT = wq[:, k % 6, ct * 128:(ct + 1) * 128]
                    for ns, nw in nsl:
                        nc.tensor.matmul(
                            psz[:, ns:ns + nw], lhsT, h[:, k, ns:ns + nw],
                            start=(k == 0), stop=(k == FT - 1),
                        )
                ysb = pys.tile([128, capc], f32, tag="ysb", name=f"ys{e}_{ct}")
                nc.vector.tensor_copy(ysb, psz)
                # y^T writes ride the (otherwise idle) gpsimd queue so
                # they never contend with the input ring for queue slots
                nc.gpsimd.dma_start(
                    out=yt_d.ap()[
                        :, yoff[e] + ct * capc:yoff[e] + (ct + 1) * capc
                    ],
                    in_=ysb,
                )

        for e in order:
            xgt, w1c, w2c = emit_loads(e)
            h = ph.tile([128, FT, capcs[e]], bf16, tag="h", name=f"h{e}")
            emit_mm1(e, xgt, w1c, h)
            emit_mm2(e, h, w2c)

    nc.compile()
    return nc


def _route(x, router_w):
    """Host routing on the full batch: per-expert global token lists,
    combine weights, balanced per-core chunks."""
    x = np.asarray(x, np.float32).reshape(N_TOK, C)
    rw = np.asarray(router_w, np.float32)
    lg = x @ rw.T                                          # [N, E]
    order2 = np.argsort(-lg, axis=-1, kind="stable")[:, :TOPK]
    m1 = np.take_along_axis(lg, order2[:, 0:1], axis=-1)[:, 0]
    m2 = np.take_along_axis(lg, order2[:, 1:2], axis=-1)[:, 0]
    g1 = 1.0 / (1.0 + np.exp((m2 - m1).astype(np.float64)))
    wts = np.stack([g1, 1.0 - g1], axis=-1).astype(np.float32)  # [N, 2]

    glists, wlists = [], []
    for e in range(E):
        sel = order2 == e                                  # [N, 2]
        any_ = sel.any(-1)
        toks = np.nonzero(any_)[0]
        w = wts[any_][sel[any_]]
        glists.append(toks)
        wlists.append(w.astype(np.float32))
    return glists, wlists


def host_prep(x, router_w, w1, b1, w2, b2, routing=None):
    """Balanced shard + lay out inputs for the 8 cores. Returns
    (in_maps, meta); meta drives the host-side combine in assemble()."""
    from ml_dtypes import bfloat16

    x = np.asarray(x, np.float32).reshape(N_TOK, C)
    router_w = np.asarray(router_w, np.float32)
    w1 = np.asarray(w1, np.float32)
    b1 = np.asarray(b1, np.float32)
    w2 = np.asarray(w2, np.float32)

    if routing is None:
        routing = _route(x, router_w)
    glists, wlists = routing
    chunks = [np.array_split(np.arange(len(glists[e])), B) for e in range(E)]
    capcs = [
        int((max(len(c) for c in chunks[e]) + 7) // 8 * 8) for e in range(E)
    ]
    order = sorted(range(E), key=lambda e: -capcs[e])

    # contiguous per-partition weight layouts:
    # w1p[e, p, q*2304 + k*384 + f'] = w1[e, k*128+p, q*384+f']
    w1b = w1.astype(bfloat16)
    w1p = np.ascontiguousarray(
        w1b.reshape(E, KC1, 128, W1CH, W1W).transpose(0, 2, 3, 1, 4)
        .reshape(E, 128, KC1 * F)
    )
    # w2p[e, p, q*4608 + kk*768 + c] = w2[e, (q*6+kk)*128+p, c]
    w2b = w2.astype(bfloat16)
    w2p = np.ascontiguousarray(
        w2b.reshape(E, W2CH, 6, 128, C).transpose(0, 3, 1, 2, 4)
        .reshape(E, 128, FT * C)
    )
    b1r = np.ascontiguousarray(
        b1.reshape(E, FT, 128).transpose(2, 0, 1).reshape(128, E * FT)
    )
    shared = {"w1p": w1p, "w2p": w2p, "b1r": b1r}

    xb = x.astype(bfloat16)
    GN6 = 6 * sum(capcs)
    maps = []
    for core in range(B):
        xg = np.zeros((128, GN6), bfloat16)
        xo = 0
        for e in order:
            capc = capcs[e]
            idx = chunks[e][core]
            toks = glists[e][idx]
            n = len(toks)
            g = np.zeros((capc, KC1, 128), bfloat16)
            g[:n] = xb[toks].reshape(n, KC1, 128)
            xg[:, xo:xo + 6 * capc] = (
                g.transpose(2, 1, 0).reshape(128, 6 * capc)
            )
            xo += 6 * capc
        maps.append({"xg": xg, **shared})

    meta = {
        "capcs": tuple(capcs),
        "order": tuple(order),
        "glists": glists,
        "wlists": wlists,
        "chunks": chunks,
    }
    return maps, meta


def assemble(res, meta, b2):
    """Host combine: out[tok] += w_e * y_e^T (transposed back), plus the
    exact b2 contribution (sum of the token's two gate weights is 1)."""
    capcs, order = meta["capcs"], meta["order"]
    glists, wlists, chunks = meta["glists"], meta["wlists"], meta["chunks"]
    out = np.zeros((N_TOK, C), np.float32)
    for core in range(B):
        yt = res.results[core]["yt"]
        yo = 0
        for e in order:
            capc = capcs[e]
            idx = chunks[e][core]
            n = len(idx)
            if n:
                y = (
                    yt[:, yo:yo + 6 * capc]
                    .reshape(128, 6, capc)[:, :, :n]
                    .transpose(2, 1, 0)
                    .reshape(n, C)
                )
                toks = glists[e][idx]
                out[toks] += wlists[e][idx][:, None] * y
            yo += 6 * capc
    b2 = np.asarray(b2, np.float32)
    if np.any(b2):
        for e in range(E):
            out[glists[e]] += wlists[e][:, None] * b2[e]
    return out.reshape(B, T, C)


def kernel(**inputs):
    _install_ntff_hook()
    from concourse import bass_utils

    in_maps, meta = host_prep(
        inputs["x"], inputs["router_w"], inputs["w1"],
        inputs["b1"], inputs["w2"], inputs["b2"],
    )
    key = ("nc", meta["capcs"], meta["order"])
    if key not in _CACHE:
        _CACHE[key] = build_program(capcs=meta["capcs"], order=meta["order"])
    nc = _CACHE[key]

    res = bass_utils.run_bass_kernel_spmd(
        nc, in_maps, core_ids=list(range(B)), trace=False
    )
    _CACHE["nc"] = nc
    _CACHE["meta"] = meta
    return assemble(res, meta, inputs["b2"])


# revision 19
# speedup vs baseline: 1.0240x; 1.0042x over previous
"""MoE layer (E=8 experts, top-2) on 8 trn2 NeuronCores.

Strategy: expert-centric balanced sharding. The host routes (fp32
logits, top-2, sigmoid gate weights), splits each expert's global token
list into 8 near-equal chunks (one per core), and gathers the x rows
for each (core, expert) chunk into a bf16 [128, 6, capc] tile. Each
core runs the same program: for each expert, stream w1/w2 (bf16,
contiguous per-partition layout, one DMA descriptor per partition),
mm1 (bf16, fp32 accum, PSUM-bank-safe token slices), GELU+bias on the
scalar engine, then mm2 c-partitioned (stationary = w2 tile, tokens
streaming) so there is no partial-token-tile waste, and write the
transposed expert output y^T contiguously to a staging DRAM tensor.
The host applies the top-2 combine weights and scatters into the final
output (0.06% of the FLOPs). The device kernel is a pure back-to-back
matmul stream: the PE never idles after the ~5us startup and the tail
is one small DMA.
"""

import sys
import types

import numpy as np

# Problem constants (nn_MoELayer_46291157516846)
E, C, F, TOPK = 8, 768, 3072, 2
B, T = 8, 2048
N_TOK = B * T
KC1 = C // 128   # 6 contraction chunks for x @ w1
FT = F // 128    # 24 f-tiles of mm1 / contraction chunks of mm2
W1CH = 12        # w1 streamed per expert in 12 chunks of 256 f-columns
W1W = F // W1CH  # 384
W2CH = 4         # w2 streamed per expert in 4 chunks of 6 k-tiles

_CACHE = {}


def _install_ntff_hook():
    """Register the NTFF profiling hook so run_bass_kernel_spmd(trace=True)
    works in this container (antenv.axon_hooks is not shipped)."""
    if "antenv.axon_hooks" in sys.modules:
        return
    mod = types.ModuleType("antenv.axon_hooks")
    mod._hook = None
    mod.set_axon_ntff_profile_hook = lambda h: setattr(mod, "_hook", h)
    mod.get_axon_ntff_profile_hook = lambda: mod._hook
    sys.modules["antenv.axon_hooks"] = mod
    try:
        import antenv

        antenv.axon_hooks = mod
        from trn_agent_boot.trn_boot import _ntff_profile_via_ctypes

        mod.set_axon_ntff_profile_hook(
            _ntff_profile_via_ctypes("/opt/axon/libaxon_pjrt.so")
        )
    except Exception:
        pass


def _slices(capc):
    """Token slices for the moving operand. A single matmul's PSUM
    output must stay within one 2KiB bank (512 fp32), so slice at 512."""
    if capc <= 512:
        return [(0, capc)]
    return [(0, 512), (512, capc - 512)]


def build_program(capcs, order):
    """Build and compile the single-core SPMD Bass program.

    capcs: per-expert compute capacities (multiples of 16, cover the
    max chunk size across cores). order: expert emission order.
    """
    import concourse.bacc as bacc
    import concourse.mybir as mybir
    from concourse.tile import TileContext
    from concourse.tile_rust import add_dep_helper

    f32 = mybir.dt.float32
    bf16 = mybir.dt.bfloat16
    Act = mybir.ActivationFunctionType

    capcs = list(capcs)
    assert len(capcs) == E and len(order) == E
    yoff = [0] * E  # per-expert offset into the staged y^T free dim
    o = 0
    for e in order:
        yoff[e] = o
        o += 6 * capcs[e]
    GN6 = 6 * sum(capcs)

    nc = bacc.Bacc("TRN2", target_bir_lowering=False, debug=False, num_devices=8)

    xg_in = nc.dram_tensor("xg", [128, GN6], bf16, kind="ExternalInput")
    # contiguous per-partition weight layouts (one DMA descriptor per
    # partition per chunk): see host_prep for the exact element order
    w1_in = nc.dram_tensor("w1p", [E, 128, KC1 * F], bf16, kind="ExternalInput")
    w2_in = nc.dram_tensor("w2p", [E, 128, FT * C], bf16, kind="ExternalInput")
    b1_in = nc.dram_tensor("b1r", [128, E * FT], f32, kind="ExternalInput")
    yt_d = nc.dram_tensor("yt", [128, GN6], f32, kind="ExternalOutput")

    from contextlib import ExitStack

    with TileContext(nc) as tc, ExitStack() as ctx:
        consts = ctx.enter_context(tc.tile_pool(name="consts", bufs=1))
        ppA = ctx.enter_context(tc.tile_pool(name="ppA", bufs=2, space="PSUM"))
        ppB = ctx.enter_context(tc.tile_pool(name="ppB", bufs=2, space="PSUM"))
        pw1 = ctx.enter_context(tc.tile_pool(name="pw1", bufs=18))
        pw2 = ctx.enter_context(tc.tile_pool(name="pw2", bufs=5))
        pxg = ctx.enter_context(tc.tile_pool(name="pxg", bufs=2))
        ph = ctx.enter_context(tc.tile_pool(name="ph", bufs=2))
        pys = ctx.enter_context(tc.tile_pool(name="pys", bufs=3))

        # ---------- small tables (scalar HWDGE queue, land first) ----------
        b1_sb = consts.tile([128, E, FT], f32)
        nc.scalar.dma_start(
            out=b1_sb, in_=b1_in.ap().rearrange("p (e t) -> p e t", e=E)
        )

        # ---------- PE warm-up during the initial DMA wait ----------
        z128 = consts.tile([128, 128], bf16)
        nc.vector.memset(z128, 0.0)
        warm = ppB.tile([128, 128], f32, tag="py", name="warm")
        for _ in range(32):
            nc.tensor.matmul(warm, z128, z128, start=True, stop=True)

        W1C = KC1 * W1W   # 2304 elements per w1 chunk per partition
        W2C = 6 * C       # 4608 elements per w2 chunk per partition

        # the input ring (sync queue) is strictly chained in consumption
        # order — the tile scheduler otherwise reorders the queue and a
        # late-needed w2 block can starve the critical w1 stream
        chain = [None]

        def ring_dma(out, in_):
            ins = nc.sync.dma_start(out=out, in_=in_)
            if chain[0] is not None:
                add_dep_helper(
                    ins.ins, chain[0].ins, sync=False,
                    reason="input ring consumption order",
                )
            chain[0] = ins
            return ins

        def emit_loads(e):
            capc = capcs[e]
            xgt = pxg.tile([128, 6, capc], bf16, tag="xg", name=f"xg{e}")
            ring_dma(
                xgt,
                xg_in.ap()[:, yoff[e]:yoff[e] + 6 * capc].rearrange(
                    "p (k t) -> p k t", k=6
                ),
            )
            w1c = []
            for q in range(W1CH):
                t = pw1.tile([128, 6, W1W], bf16, tag="w1", name=f"w1_{e}_{q}")
                ring_dma(
                    t,
                    w1_in.ap()[e][:, q * W1C:(q + 1) * W1C].rearrange(
                        "p (k f) -> p k f", k=6
                    ),
                )
                w1c.append(t)
            w2c = []
            for q in range(W2CH):
                t = pw2.tile([128, 6, C], bf16, tag="w2", name=f"w2_{e}_{q}")
                ring_dma(
                    t,
                    w2_in.ap()[e][:, q * W2C:(q + 1) * W2C].rearrange(
                        "p (k c) -> p k c", k=6
                    ),
                )
                w2c.append(t)
            return xgt, w1c, w2c

        FPC = W1W // 128  # f-tiles per w1 chunk

        def emit_mm1(e, xgt, w1c, h):
            capc = capcs[e]
            nsl = _slices(capc)
            for ft in range(FT):
                wt = w1c[ft // FPC]
                fc = (ft % FPC) * 128
                psh = ppA.tile([128, capc], f32, tag="pp", name=f"psh{e}_{ft}")
                for k in range(KC1):
                    lhsT = wt[:, k, fc:fc + 128]
                    for ns, nw in nsl:
                        nc.tensor.matmul(
                            psh[:, ns:ns + nw], lhsT, xgt[:, k, ns:ns + nw],
                            start=(k == 0), stop=(k == KC1 - 1),
                        )
                nc.scalar.activation(
                    h[:, ft, :], psh, Act.Gelu,
                    bias=b1_sb[:, e, ft:ft + 1], scale=1.0,
                )

        def emit_mm2(e, h, w2c):
            """mm2 c-partitioned: out^T[c_tile, tok] += w2_chunk.T @ h_chunk.
            No partial-token-tile waste; y^T goes straight to DRAM."""
            capc = capcs[e]
            nsl = _slices(capc)
            for ct in range(KC1):
                psz = ppB.tile([128, capc], f32, tag="py", name=f"psz{e}_{ct}")
                for k in range(FT):
                    wq = w2c[k // 6]
                    lhsT = wq[:, k % 6, ct * 128:(ct + 1) * 128]
                    for ns, nw in nsl:
                        nc.tensor.matmul(
                            psz[:, ns:ns + nw], lhsT, h[:, k, ns:ns + nw],
                            start=(k == 0), stop=(k == FT - 1),
                        )
                ysb = pys.tile([128, capc], f32, tag="ysb", name=f"ys{e}_{ct}")
                nc.vector.tensor_copy(ysb, psz)
                # y^T writes ride the scalar queue (activations have
                # slack) so they never reorder the input ring
                nc.scalar.dma_start(
                    out=yt_d.ap()[
                        :, yoff[e] + ct * capc:yoff[e] + (ct + 1) * capc
                    ],
                    in_=ysb,
                )

        for e in order:
            xgt, w1c, w2c = emit_loads(e)
            h = ph.tile([128, FT, capcs[e]], bf16, tag="h", name=f"h{e}")
            emit_mm1(e, xgt, w1c, h)
            emit_mm2(e, h, w2c)

    nc.compile()
    return nc


def _route(x, router_w):
    """Host routing on the full batch: per-expert global token lists,
    combine weights, balanced per-core chunks."""
    x = np.asarray(x, np.float32).reshape(N_TOK, C)
    rw = np.asarray(router_w, np.float32)
    lg = x @ rw.T                                          # [N, E]
    order2 = np.argsort(-lg, axis=-1, kind="stable")[:, :TOPK]
    m1 = np.take_along_axis(lg, order2[:, 0:1], axis=-1)[:, 0]
    m2 = np.take_along_axis(lg, order2[:, 1:2], axis=-1)[:, 0]
    g1 = 1.0 / (1.0 + np.exp((m2 - m1).astype(np.float64)))
    wts = np.stack([g1, 1.0 - g1], axis=-1).astype(np.float32)  # [N, 2]

    glists, wlists = [], []
    for e in range(E):
        sel = order2 == e                                  # [N, 2]
        any_ = sel.any(-1)
        toks = np.nonzero(any_)[0]
        w = wts[any_][sel[any_]]
        glists.append(toks)
        wlists.append(w.astype(np.float32))
    return glists, wlists


def host_prep(x, router_w, w1, b1, w2, b2, routing=None):
    """Balanced shard + lay out inputs for the 8 cores. Returns
    (in_maps, meta); meta drives the host-side combine in assemble()."""
    from ml_dtypes import bfloat16

    x = np.asarray(x, np.float32).reshape(N_TOK, C)
    router_w = np.asarray(router_w, np.float32)
    w1 = np.asarray(w1, np.float32)
    b1 = np.asarray(b1, np.float32)
    w2 = np.asarray(w2, np.float32)

    if routing is None:
        routing = _route(x, router_w)
    glists, wlists = routing
    chunks = [np.array_split(np.arange(len(glists[e])), B) for e in range(E)]
    capcs = [
        int((max(len(c) for c in chunks[e]) + 7) // 8 * 8) for e in range(E)
    ]
    order = sorted(range(E), key=lambda e: -capcs[e])

    # contiguous per-partition weight layouts:
    # w1p[e, p, q*2304 + k*384 + f'] = w1[e, k*128+p, q*384+f']
    w1b = w1.astype(bfloat16)
    w1p = np.ascontiguousarray(
        w1b.reshape(E, KC1, 128, W1CH, W1W).transpose(0, 2, 3, 1, 4)
        .reshape(E, 128, KC1 * F)
    )
    # w2p[e, p, q*4608 + kk*768 + c] = w2[e, (q*6+kk)*128+p, c]
    w2b = w2.astype(bfloat16)
    w2p = np.ascontiguousarray(
        w2b.reshape(E, W2CH, 6, 128, C).transpose(0, 3, 1, 2, 4)
        .reshape(E, 128, FT * C)
    )
    b1r = np.ascontiguousarray(
        b1.reshape(E, FT, 128).transpose(2, 0, 1).reshape(128, E * FT)
    )
    shared = {"w1p": w1p, "w2p": w2p, "b1r": b1r}

    xb = x.astype(bfloat16)
    GN6 = 6 * sum(capcs)
    maps = []
    for core in range(B):
        xg = np.zeros((128, GN6), bfloat16)
        xo = 0
        for e in order:
            capc = capcs[e]
            idx = chunks[e][core]
            toks = glists[e][idx]
            n = len(toks)
            g = np.zeros((capc, KC1, 128), bfloat16)
            g[:n] = xb[toks].reshape(n, KC1, 128)
            xg[:, xo:xo + 6 * capc] = (
                g.transpose(2, 1, 0).reshape(128, 6 * capc)
            )
            xo += 6 * capc
        maps.append({"xg": xg, **shared})

    meta = {
        "capcs": tuple(capcs),
        "order": tuple(order),
        "glists": glists,
        "wlists": wlists,
        "chunks": chunks,
    }
    return maps, meta


def assemble(res, meta, b2):
    """Host combine: out[tok] += w_e * y_e^T (transposed back), plus the
    exact b2 contribution (sum of the token's two gate weights is 1)."""
    capcs, order = meta["capcs"], meta["order"]
    glists, wlists, chunks = meta["glists"], meta["wlists"], meta["chunks"]
    out = np.zeros((N_TOK, C), np.float32)
    for core in range(B):
        yt = res.results[core]["yt"]
        yo = 0
        for e in order:
            capc = capcs[e]
            idx = chunks[e][core]
            n = len(idx)
            if n:
                y = (
                    yt[:, yo:yo + 6 * capc]
                    .reshape(128, 6, capc)[:, :, :n]
                    .transpose(2, 1, 0)
                    .reshape(n, C)
                )
                toks = glists[e][idx]
                out[toks] += wlists[e][idx][:, None] * y
            yo += 6 * capc
    b2 = np.asarray(b2, np.float32)
    if np.any(b2):
        for e in range(E):
            out[glists[e]] += wlists[e][:, None] * b2[e]
    return out.reshape(B, T, C)


def kernel(**inputs):
    _install_ntff_hook()
    from concourse import bass_utils

    in_maps, meta = host_prep(
        inputs["x"], inputs["router_w"], inputs["w1"],
        inputs["b1"], inputs["w2"], inputs["b2"],
    )
    key = ("nc", meta["capcs"], meta["order"])
    if key not in _CACHE:
        _CACHE[key] = build_program(capcs=meta["capcs"], order=meta["order"])
    nc = _CACHE[key]

    res = bass_utils.run_bass_kernel_spmd(
        nc, in_maps, core_ids=list(range(B)), trace=False
    )
    _CACHE["nc"] = nc
    _CACHE["meta"] = meta
    return assemble(res, meta, inputs["b2"])


# revision 20
# speedup vs baseline: 1.0280x; 1.0039x over previous
"""MoE layer (E=8 experts, top-2) on 8 trn2 NeuronCores.

Strategy: expert-centric balanced sharding. The host routes (fp32
logits, top-2, sigmoid gate weights), splits each expert's global token
list into 8 near-equal chunks (one per core), and gathers the x rows
for each (core, expert) chunk into a bf16 [128, 6, capc] tile. Each
core runs the same program: for each expert, stream w1/w2 (bf16,
contiguous per-partition layout, one DMA descriptor per partition),
mm1 (bf16, fp32 accum, PSUM-bank-safe token slices), GELU+bias on the
scalar engine, then mm2 c-partitioned (stationary = w2 tile, tokens
streaming) so there is no partial-token-tile waste, and write the
transposed expert output y^T contiguously to a staging DRAM tensor.
The host applies the top-2 combine weights and scatters into the final
output (0.06% of the FLOPs). The device kernel is a pure back-to-back
matmul stream: the PE never idles after the ~5us startup and the tail
is one small DMA.
"""

import sys
import types

import numpy as np

# Problem constants (nn_MoELayer_46291157516846)
E, C, F, TOPK = 8, 768, 3072, 2
B, T = 8, 2048
N_TOK = B * T
KC1 = C // 128   # 6 contraction chunks for x @ w1
FT = F // 128    # 24 f-tiles of mm1 / contraction chunks of mm2
W1CH = 12        # w1 streamed per expert in 12 chunks of 256 f-columns
W1W = F // W1CH  # 384
W2CH = 4         # w2 streamed per expert in 4 chunks of 6 k-tiles

_CACHE = {}


def _install_ntff_hook():
    """Register the NTFF profiling hook so run_bass_kernel_spmd(trace=True)
    works in this container (antenv.axon_hooks is not shipped)."""
    if "antenv.axon_hooks" in sys.modules:
        return
    mod = types.ModuleType("antenv.axon_hooks")
    mod._hook = None
    mod.set_axon_ntff_profile_hook = lambda h: setattr(mod, "_hook", h)
    mod.get_axon_ntff_profile_hook = lambda: mod._hook
    sys.modules["antenv.axon_hooks"] = mod
    try:
        import antenv

        antenv.axon_hooks = mod
        from trn_agent_boot.trn_boot import _ntff_profile_via_ctypes

        mod.set_axon_ntff_profile_hook(
            _ntff_profile_via_ctypes("/opt/axon/libaxon_pjrt.so")
        )
    except Exception:
        pass


def _slices(capc):
    """Token slices for the moving operand. A single matmul's PSUM
    output must stay within one 2KiB bank (512 fp32), so slice at 512."""
    if capc <= 512:
        return [(0, capc)]
    return [(0, 512), (512, capc - 512)]


def build_program(capcs, order):
    """Build and compile the single-core SPMD Bass program.

    capcs: per-expert compute capacities (multiples of 16, cover the
    max chunk size across cores). order: expert emission order.
    """
    import concourse.bacc as bacc
    import concourse.mybir as mybir
    from concourse.tile import TileContext
    from concourse.tile_rust import add_dep_helper

    f32 = mybir.dt.float32
    bf16 = mybir.dt.bfloat16
    Act = mybir.ActivationFunctionType

    capcs = list(capcs)
    assert len(capcs) == E and len(order) == E
    yoff = [0] * E  # per-expert offset into the staged y^T free dim
    o = 0
    for e in order:
        yoff[e] = o
        o += 6 * capcs[e]
    GN6 = 6 * sum(capcs)

    nc = bacc.Bacc("TRN2", target_bir_lowering=False, debug=False, num_devices=8)

    xg_in = nc.dram_tensor("xg", [128, GN6], bf16, kind="ExternalInput")
    # contiguous per-partition weight layouts (one DMA descriptor per
    # partition per chunk): see host_prep for the exact element order
    w1_in = nc.dram_tensor("w1p", [E, 128, KC1 * F], bf16, kind="ExternalInput")
    w2_in = nc.dram_tensor("w2p", [E, 128, FT * C], bf16, kind="ExternalInput")
    b1_in = nc.dram_tensor("b1r", [128, E * FT], f32, kind="ExternalInput")
    yt_d = nc.dram_tensor("yt", [128, GN6], f32, kind="ExternalOutput")

    from contextlib import ExitStack

    with TileContext(nc) as tc, ExitStack() as ctx:
        consts = ctx.enter_context(tc.tile_pool(name="consts", bufs=1))
        ppA = ctx.enter_context(tc.tile_pool(name="ppA", bufs=2, space="PSUM"))
        ppB = ctx.enter_context(tc.tile_pool(name="ppB", bufs=2, space="PSUM"))
        pw1 = ctx.enter_context(tc.tile_pool(name="pw1", bufs=18))
        pw2 = ctx.enter_context(tc.tile_pool(name="pw2", bufs=5))
        pxg = ctx.enter_context(tc.tile_pool(name="pxg", bufs=2))
        ph = ctx.enter_context(tc.tile_pool(name="ph", bufs=2))
        pys = ctx.enter_context(tc.tile_pool(name="pys", bufs=3))

        # ---------- small tables (scalar HWDGE queue, land first) ----------
        b1_sb = consts.tile([128, E, FT], f32)
        nc.scalar.dma_start(
            out=b1_sb, in_=b1_in.ap().rearrange("p (e t) -> p e t", e=E)
        )

        # ---------- PE warm-up during the initial DMA wait ----------
        z128 = consts.tile([128, 128], bf16)
        nc.vector.memset(z128, 0.0)
        warm = ppB.tile([128, 128], f32, tag="py", name="warm")
        for _ in range(48):
            nc.tensor.matmul(warm, z128, z128, start=True, stop=True)

        W1C = KC1 * W1W   # 2304 elements per w1 chunk per partition
        W2C = 6 * C       # 4608 elements per w2 chunk per partition

        # the input ring (sync queue) is strictly chained in consumption
        # order — the tile scheduler otherwise reorders the queue and a
        # late-needed w2 block can starve the critical w1 stream
        chain = [None]

        def ring_dma(out, in_):
            ins = nc.sync.dma_start(out=out, in_=in_)
            if chain[0] is not None:
                add_dep_helper(
                    ins.ins, chain[0].ins, sync=False,
                    reason="input ring consumption order",
                )
            chain[0] = ins
            return ins

        def emit_loads(e):
            capc = capcs[e]
            xgt = pxg.tile([128, 6, capc], bf16, tag="xg", name=f"xg{e}")
            ring_dma(
                xgt,
                xg_in.ap()[:, yoff[e]:yoff[e] + 6 * capc].rearrange(
                    "p (k t) -> p k t", k=6
                ),
            )
            w1c = []
            for q in range(W1CH):
                t = pw1.tile([128, 6, W1W], bf16, tag="w1", name=f"w1_{e}_{q}")
                ring_dma(
                    t,
                    w1_in.ap()[e][:, q * W1C:(q + 1) * W1C].rearrange(
                        "p (k f) -> p k f", k=6
                    ),
                )
                w1c.append(t)
            w2c = []
            for q in range(W2CH):
                t = pw2.tile([128, 6, C], bf16, tag="w2", name=f"w2_{e}_{q}")
                ring_dma(
                    t,
                    w2_in.ap()[e][:, q * W2C:(q + 1) * W2C].rearrange(
                        "p (k c) -> p k c", k=6
                    ),
                )
                w2c.append(t)
            return xgt, w1c, w2c

        FPC = W1W // 128  # f-tiles per w1 chunk

        def emit_mm1(e, xgt, w1c, h):
            capc = capcs[e]
            nsl = _slices(capc)
            for ft in range(FT):
                wt = w1c[ft // FPC]
                fc = (ft % FPC) * 128
                psh = ppA.tile([128, capc], f32, tag="pp", name=f"psh{e}_{ft}")
                for k in range(KC1):
                    lhsT = wt[:, k, fc:fc + 128]
                    for ns, nw in nsl:
                        nc.tensor.matmul(
                            psh[:, ns:ns + nw], lhsT, xgt[:, k, ns:ns + nw],
                            start=(k == 0), stop=(k == KC1 - 1),
                        )
                nc.scalar.activation(
                    h[:, ft, :], psh, Act.Gelu,
                    bias=b1_sb[:, e, ft:ft + 1], scale=1.0,
                )

        def emit_mm2(e, h, w2c):
            """mm2 c-partitioned: out^T[c_tile, tok] += w2_chunk.T @ h_chunk.
            No partial-token-tile waste; y^T goes straight to DRAM."""
            capc = capcs[e]
            nsl = _slices(capc)
            for ct in range(KC1):
                psz = ppB.tile([128, capc], f32, tag="py", name=f"psz{e}_{ct}")
                for k in range(FT):
                    wq = w2c[k // 6]
                    lhsT = wq[:, k % 6, ct * 128:(ct + 1) * 128]
                    for ns, nw in nsl:
                        nc.tensor.matmul(
                            psz[:, ns:ns + nw], lhsT, h[:, k, ns:ns + nw],
                            start=(k == 0), stop=(k == FT - 1),
                        )
                ysb = pys.tile([128, capc], f32, tag="ysb", name=f"ys{e}_{ct}")
                nc.vector.tensor_copy(ysb, psz)
                # y^T writes ride the scalar queue (activations have
                # slack) so they never reorder the input ring
                nc.scalar.dma_start(
                    out=yt_d.ap()[
                        :, yoff[e] + ct * capc:yoff[e] + (ct + 1) * capc
                    ],
                    in_=ysb,
                )

        for e in order:
            xgt, w1c, w2c = emit_loads(e)
            h = ph.tile([128, FT, capcs[e]], bf16, tag="h", name=f"h{e}")
            emit_mm1(e, xgt, w1c, h)
            emit_mm2(e, h, w2c)

    nc.compile()
    return nc


def _route(x, router_w):
    """Host routing on the full batch: per-expert global token lists,
    combine weights, balanced per-core chunks."""
    x = np.asarray(x, np.float32).reshape(N_TOK, C)
    rw = np.asarray(router_w, np.float32)
    lg = x @ rw.T                                          # [N, E]
    order2 = np.argsort(-lg, axis=-1, kind="stable")[:, :TOPK]
    m1 = np.take_along_axis(lg, order2[:, 0:1], axis=-1)[:, 0]
    m2 = np.take_along_axis(lg, order2[:, 1:2], axis=-1)[:, 0]
    g1 = 1.0 / (1.0 + np.exp((m2 - m1).astype(np.float64)))
    wts = np.stack([g1, 1.0 - g1], axis=-1).astype(np.float32)  # [N, 2]

    glists, wlists = [], []
    for e in range(E):
        sel = order2 == e                                  # [N, 2]
        any_ = sel.any(-1)
        toks = np.nonzero(any_)[0]
        w = wts[any_][sel[any_]]
        glists.append(toks)
        wlists.append(w.astype(np.float32))
    return glists, wlists


def host_prep(x, router_w, w1, b1, w2, b2, routing=None):
    """Balanced shard + lay out inputs for the 8 cores. Returns
    (in_maps, meta); meta drives the host-side combine in assemble()."""
    from ml_dtypes import bfloat16

    x = np.asarray(x, np.float32).reshape(N_TOK, C)
    router_w = np.asarray(router_w, np.float32)
    w1 = np.asarray(w1, np.float32)
    b1 = np.asarray(b1, np.float32)
    w2 = np.asarray(w2, np.float32)

    if routing is None:
        routing = _route(x, router_w)
    glists, wlists = routing
    chunks = [np.array_split(np.arange(len(glists[e])), B) for e in range(E)]
    capcs = [
        int((max(len(c) for c in chunks[e]) + 7) // 8 * 8) for e in range(E)
    ]
    order = sorted(range(E), key=lambda e: -capcs[e])

    # contiguous per-partition weight layouts:
    # w1p[e, p, q*2304 + k*384 + f'] = w1[e, k*128+p, q*384+f']
    w1b = w1.astype(bfloat16)
    w1p = np.ascontiguousarray(
        w1b.reshape(E, KC1, 128, W1CH, W1W).transpose(0, 2, 3, 1, 4)
        .reshape(E, 128, KC1 * F)
    )
    # w2p[e, p, q*4608 + kk*768 + c] = w2[e, (q*6+kk)*128+p, c]
    w2b = w2.astype(bfloat16)
    w2p = np.ascontiguousarray(
        w2b.reshape(E, W2CH, 6, 128, C).transpose(0, 3, 1, 2, 4)
        .reshape(E, 128, FT * C)
    )
    b1r = np.ascontiguousarray(
        b1.reshape(E, FT, 128).transpose(2, 0, 1).reshape(128, E * FT)
    )
    shared = {"w1p": w1p, "w2p": w2p, "b1r": b1r}

    xb = x.astype(bfloat16)
    GN6 = 6 * sum(capcs)
    maps = []
    for core in range(B):
        xg = np.zeros((128, GN6), bfloat16)
        xo = 0
        for e in order:
            capc = capcs[e]
            idx = chunks[e][core]
            toks = glists[e][idx]
            n = len(toks)
            g = np.zeros((capc, KC1, 128), bfloat16)
            g[:n] = xb[toks].reshape(n, KC1, 128)
            xg[:, xo:xo + 6 * capc] = (
                g.transpose(2, 1, 0).reshape(128, 6 * capc)
            )
            xo += 6 * capc
        maps.append({"xg": xg, **shared})

    meta = {
        "capcs": tuple(capcs),
        "order": tuple(order),
        "glists": glists,
        "wlists": wlists,
        "chunks": chunks,
    }
    return maps, meta


def assemble(res, meta, b2):
    """Host combine: out[tok] += w_e * y_e^T (transposed back), plus the
    exact b2 contribution (sum of the token's two gate weights is 1)."""
    capcs, order = meta["capcs"], meta["order"]
    glists, wlists, chunks = meta["glists"], meta["wlists"], meta["chunks"]
    out = np.zeros((N_TOK, C), np.float32)
    for core in range(B):
        yt = res.results[core]["yt"]
        yo = 0
        for e in order:
            capc = capcs[e]
            idx = chunks[e][core]
            n = len(idx)
            if n:
                y = (
                    yt[:, yo:yo + 6 * capc]
                    .reshape(128, 6, capc)[:, :, :n]
                    .transpose(2, 1, 0)
                    .reshape(n, C)
                )
                toks = glists[e][idx]
                out[toks] += wlists[e][idx][:, None] * y
            yo += 6 * capc
    b2 = np.asarray(b2, np.float32)
    if np.any(b2):
        for e in range(E):
            out[glists[e]] += wlists[e][:, None] * b2[e]
    return out.reshape(B, T, C)


def kernel(**inputs):
    _install_ntff_hook()
    from concourse import bass_utils

    in_maps, meta = host_prep(
        inputs["x"], inputs["router_w"], inputs["w1"],
        inputs["b1"], inputs["w2"], inputs["b2"],
    )
    key = ("nc", meta["capcs"], meta["order"])
    if key not in _CACHE:
        _CACHE[key] = build_program(capcs=meta["capcs"], order=meta["order"])
    nc = _CACHE[key]

    res = bass_utils.run_bass_kernel_spmd(
        nc, in_maps, core_ids=list(range(B)), trace=False
    )
    _CACHE["nc"] = nc
    _CACHE["meta"] = meta
    return assemble(res, meta, inputs["b2"])


# revision 24
# speedup vs baseline: 1.0685x; 1.0394x over previous
"""MoE layer (E=8 experts, top-2) on 8 trn2 NeuronCores.

Strategy: expert-centric balanced sharding. The host routes (fp32
logits, top-2, sigmoid gate weights), splits each expert's global token
list into 8 near-equal chunks (one per core), and gathers the x rows
for each (core, expert) chunk into a bf16 [128, 6, capc] tile. Each
core runs the same program: for each expert, stream w1/w2 (bf16,
contiguous per-partition layout, one DMA descriptor per partition),
mm1 (bf16, fp32 accum, PSUM-bank-safe token slices), GELU+bias on the
scalar engine, then mm2 c-partitioned (stationary = w2 tile, tokens
streaming) so there is no partial-token-tile waste, and write the
transposed expert output y^T contiguously to a staging DRAM tensor.
The host applies the top-2 combine weights and scatters into the final
output (0.06% of the FLOPs). The device kernel is a pure back-to-back
matmul stream: the PE never idles after the ~5us startup and the tail
is one small DMA.
"""

import sys
import types

import numpy as np

# Problem constants (nn_MoELayer_46291157516846)
E, C, F, TOPK = 8, 768, 3072, 2
B, T = 8, 2048
N_TOK = B * T
KC1 = C // 128   # 6 contraction chunks for x @ w1
FT = F // 128    # 24 f-tiles of mm1 / contraction chunks of mm2
W1CH = 12        # w1 streamed per expert in 12 chunks of 256 f-columns
W1W = F // W1CH  # 384
W2CH = 4         # w2 streamed per expert in 4 chunks of 6 k-tiles

_CACHE = {}


def _install_ntff_hook():
    """Register the NTFF profiling hook so run_bass_kernel_spmd(trace=True)
    works in this container (antenv.axon_hooks is not shipped)."""
    if "antenv.axon_hooks" in sys.modules:
        return
    mod = types.ModuleType("antenv.axon_hooks")
    mod._hook = None
    mod.set_axon_ntff_profile_hook = lambda h: setattr(mod, "_hook", h)
    mod.get_axon_ntff_profile_hook = lambda: mod._hook
    sys.modules["antenv.axon_hooks"] = mod
    try:
        import antenv

        antenv.axon_hooks = mod
        from trn_agent_boot.trn_boot import _ntff_profile_via_ctypes

        mod.set_axon_ntff_profile_hook(
            _ntff_profile_via_ctypes("/opt/axon/libaxon_pjrt.so")
        )
    except Exception:
        pass


def _slices(capc):
    """(psum_offset, [(psum_col, width), ...]) for the moving operand.

    A single matmul's PSUM output must stay inside one 2KiB bank
    (512 fp32). A remainder slice under ~60 tokens would pay the
    per-matmul issue floor, so when 0 < capc-512 < 64 the accumulator
    region is shifted 64 columns into the (bank-aligned) tile: the
    split becomes (448, capc-448) and both slices clear the floor.
    Token s lives at psum column off + s."""
    if capc <= 512:
        return 0, [(0, capc)]
    if capc - 512 >= 64:
        return 0, [(0, 512), (512, capc - 512)]
    off = 64
    return off, [(off, 512 - off), (512, capc + off - 512)]


def build_program(capcs, order):
    """Build and compile the single-core SPMD Bass program.

    capcs: per-expert compute capacities (multiples of 16, cover the
    max chunk size across cores). order: expert emission order.
    """
    import concourse.bacc as bacc
    import concourse.mybir as mybir
    from concourse.tile import TileContext
    from concourse.tile_rust import add_dep_helper

    f32 = mybir.dt.float32
    bf16 = mybir.dt.bfloat16
    Act = mybir.ActivationFunctionType

    capcs = list(capcs)
    assert len(capcs) == E and len(order) == E
    yoff = [0] * E  # per-expert offset into the staged y^T free dim
    o = 0
    for e in order:
        yoff[e] = o
        o += 6 * capcs[e]
    GN6 = 6 * sum(capcs)

    nc = bacc.Bacc("TRN2", target_bir_lowering=False, debug=False, num_devices=8)

    xg_in = nc.dram_tensor("xg", [128, GN6], bf16, kind="ExternalInput")
    # contiguous per-partition weight layouts (one DMA descriptor per
    # partition per chunk): see host_prep for the exact element order
    w1_in = nc.dram_tensor("w1p", [E, 128, KC1 * F], bf16, kind="ExternalInput")
    w2_in = nc.dram_tensor("w2p", [E, 128, FT * C], bf16, kind="ExternalInput")
    b1_in = nc.dram_tensor("b1r", [128, E * FT], f32, kind="ExternalInput")
    yt_d = nc.dram_tensor("yt", [128, GN6], f32, kind="ExternalOutput")

    from contextlib import ExitStack

    with TileContext(nc) as tc, ExitStack() as ctx:
        consts = ctx.enter_context(tc.tile_pool(name="consts", bufs=1))
        ppA = ctx.enter_context(tc.tile_pool(name="ppA", bufs=2, space="PSUM"))
        ppB = ctx.enter_context(tc.tile_pool(name="ppB", bufs=2, space="PSUM"))
        pw1 = ctx.enter_context(tc.tile_pool(name="pw1", bufs=18))
        pw2 = ctx.enter_context(tc.tile_pool(name="pw2", bufs=5))
        pxg = ctx.enter_context(tc.tile_pool(name="pxg", bufs=2))
        ph = ctx.enter_context(tc.tile_pool(name="ph", bufs=2))
        pys = ctx.enter_context(tc.tile_pool(name="pys", bufs=3))

        # ---------- small tables (scalar HWDGE queue, land first) ----------
        b1_sb = consts.tile([128, E, FT], f32)
        nc.scalar.dma_start(
            out=b1_sb, in_=b1_in.ap().rearrange("p (e t) -> p e t", e=E)
        )

        # ---------- PE warm-up during the initial DMA wait ----------
        z128 = consts.tile([128, 128], bf16)
        nc.vector.memset(z128, 0.0)
        warm = ppB.tile([128, 128], f32, tag="py", name="warm")
        for _ in range(48):
            nc.tensor.matmul(warm, z128, z128, start=True, stop=True)

        W1C = KC1 * W1W   # 2304 elements per w1 chunk per partition
        W2C = 6 * C       # 4608 elements per w2 chunk per partition

        # the input ring (sync queue) is strictly chained in consumption
        # order — the tile scheduler otherwise reorders the queue and a
        # late-needed w2 block can starve the critical w1 stream
        chain = [None]

        def ring_dma(out, in_):
            ins = nc.sync.dma_start(out=out, in_=in_)
            if chain[0] is not None:
                add_dep_helper(
                    ins.ins, chain[0].ins, sync=False,
                    reason="input ring consumption order",
                )
            chain[0] = ins
            return ins

        def emit_loads(e):
            capc = capcs[e]
            xgt = pxg.tile([128, 6, capc], bf16, tag="xg", name=f"xg{e}")
            ring_dma(
                xgt,
                xg_in.ap()[:, yoff[e]:yoff[e] + 6 * capc].rearrange(
                    "p (k t) -> p k t", k=6
                ),
            )
            w1c = []
            for q in range(W1CH):
                t = pw1.tile([128, 6, W1W], bf16, tag="w1", name=f"w1_{e}_{q}")
                ring_dma(
                    t,
                    w1_in.ap()[e][:, q * W1C:(q + 1) * W1C].rearrange(
                        "p (k f) -> p k f", k=6
                    ),
                )
                w1c.append(t)
            w2c = []
            for q in range(W2CH):
                t = pw2.tile([128, 6, C], bf16, tag="w2", name=f"w2_{e}_{q}")
                ring_dma(
                    t,
                    w2_in.ap()[e][:, q * W2C:(q + 1) * W2C].rearrange(
                        "p (k c) -> p k c", k=6
                    ),
                )
                w2c.append(t)
            return xgt, w1c, w2c

        FPC = W1W // 128  # f-tiles per w1 chunk

        def emit_mm1(e, xgt, w1c, h):
            capc = capcs[e]
            off, nsl = _slices(capc)
            for ft in range(FT):
                wt = w1c[ft // FPC]
                fc = (ft % FPC) * 128
                psh = ppA.tile(
                    [128, capc + off], f32, tag="pp", name=f"psh{e}_{ft}"
                )
                for k in range(KC1):
                    lhsT = wt[:, k, fc:fc + 128]
                    for ns, nw in nsl:
                        nc.tensor.matmul(
                            psh[:, ns:ns + nw], lhsT,
                            xgt[:, k, ns - off:ns - off + nw],
                            start=(k == 0), stop=(k == KC1 - 1),
                        )
                nc.scalar.activation(
                    h[:, ft, :], psh[:, off:off + capc], Act.Gelu,
                    bias=b1_sb[:, e, ft:ft + 1], scale=1.0,
                )

        def emit_mm2(e, h, w2c):
            """mm2 c-partitioned: out^T[c_tile, tok] += w2_chunk.T @ h_chunk.
            No partial-token-tile waste; y^T goes straight to DRAM."""
            capc = capcs[e]
            off, nsl = _slices(capc)
            for ct in range(KC1):
                psz = ppB.tile(
                    [128, capc + off], f32, tag="py", name=f"psz{e}_{ct}"
                )
                for k in range(FT):
                    wq = w2c[k // 6]
                    lhsT = wq[:, k % 6, ct * 128:(ct + 1) * 128]
                    for ns, nw in nsl:
                        nc.tensor.matmul(
                            psz[:, ns:ns + nw], lhsT,
                            h[:, k, ns - off:ns - off + nw],
                            start=(k == 0), stop=(k == FT - 1),
                        )
                ysb = pys.tile([128, capc], f32, tag="ysb", name=f"ys{e}_{ct}")
                nc.vector.tensor_copy(ysb, psz[:, off:off + capc])
                # y^T writes ride the scalar queue (activations have
                # slack) so they never reorder the input ring
                nc.scalar.dma_start(
                    out=yt_d.ap()[
                        :, yoff[e] + ct * capc:yoff[e] + (ct + 1) * capc
                    ],
                    in_=ysb,
                )

        for e in order:
            xgt, w1c, w2c = emit_loads(e)
            h = ph.tile([128, FT, capcs[e]], bf16, tag="h", name=f"h{e}")
            emit_mm1(e, xgt, w1c, h)
            emit_mm2(e, h, w2c)

    nc.compile()
    return nc


def _route(x, router_w):
    """Host routing on the full batch: per-expert global token lists,
    combine weights, balanced per-core chunks."""
    x = np.asarray(x, np.float32).reshape(N_TOK, C)
    rw = np.asarray(router_w, np.float32)
    lg = x @ rw.T                                          # [N, E]
    order2 = np.argsort(-lg, axis=-1, kind="stable")[:, :TOPK]
    m1 = np.take_along_axis(lg, order2[:, 0:1], axis=-1)[:, 0]
    m2 = np.take_along_axis(lg, order2[:, 1:2], axis=-1)[:, 0]
    g1 = 1.0 / (1.0 + np.exp((m2 - m1).astype(np.float64)))
    wts = np.stack([g1, 1.0 - g1], axis=-1).astype(np.float32)  # [N, 2]

    glists, wlists = [], []
    for e in range(E):
        sel = order2 == e                                  # [N, 2]
        any_ = sel.any(-1)
        toks = np.nonzero(any_)[0]
        w = wts[any_][sel[any_]]
        glists.append(toks)
        wlists.append(w.astype(np.float32))
    return glists, wlists


def host_prep(x, router_w, w1, b1, w2, b2, routing=None):
    """Balanced shard + lay out inputs for the 8 cores. Returns
    (in_maps, meta); meta drives the host-side combine in assemble()."""
    from ml_dtypes import bfloat16

    x = np.asarray(x, np.float32).reshape(N_TOK, C)
    router_w = np.asarray(router_w, np.float32)
    w1 = np.asarray(w1, np.float32)
    b1 = np.asarray(b1, np.float32)
    w2 = np.asarray(w2, np.float32)

    if routing is None:
        routing = _route(x, router_w)
    glists, wlists = routing
    chunks = [np.array_split(np.arange(len(glists[e])), B) for e in range(E)]
    capcs = [int(max(len(c) for c in chunks[e])) for e in range(E)]
    order = sorted(range(E), key=lambda e: -capcs[e])

    # contiguous per-partition weight layouts:
    # w1p[e, p, q*2304 + k*384 + f'] = w1[e, k*128+p, q*384+f']
    w1b = w1.astype(bfloat16)
    w1p = np.ascontiguousarray(
        w1b.reshape(E, KC1, 128, W1CH, W1W).transpose(0, 2, 3, 1, 4)
        .reshape(E, 128, KC1 * F)
    )
    # w2p[e, p, q*4608 + kk*768 + c] = w2[e, (q*6+kk)*128+p, c]
    w2b = w2.astype(bfloat16)
    w2p = np.ascontiguousarray(
        w2b.reshape(E, W2CH, 6, 128, C).transpose(0, 3, 1, 2, 4)
        .reshape(E, 128, FT * C)
    )
    b1r = np.ascontiguousarray(
        b1.reshape(E, FT, 128).transpose(2, 0, 1).reshape(128, E * FT)
    )
    shared = {"w1p": w1p, "w2p": w2p, "b1r": b1r}

    xb = x.astype(bfloat16)
    GN6 = 6 * sum(capcs)
    maps = []
    for core in range(B):
        xg = np.zeros((128, GN6), bfloat16)
        xo = 0
        for e in order:
            capc = capcs[e]
            idx = chunks[e][core]
            toks = glists[e][idx]
            n = len(toks)
            g = np.zeros((capc, KC1, 128), bfloat16)
            g[:n] = xb[toks].reshape(n, KC1, 128)
            xg[:, xo:xo + 6 * capc] = (
                g.transpose(2, 1, 0).reshape(128, 6 * capc)
            )
            xo += 6 * capc
        maps.append({"xg": xg, **shared})

    meta = {
        "capcs": tuple(capcs),
        "order": tuple(order),
        "glists": glists,
        "wlists": wlists,
        "chunks": chunks,
    }
    return maps, meta


def assemble(res, meta, b2):
    """Host combine: out[tok] += w_e * y_e^T (transposed back), plus the
    exact b2 contribution (sum of the token's two gate weights is 1)."""
    capcs, order = meta["capcs"], meta["order"]
    glists, wlists, chunks = meta["glists"], meta["wlists"], meta["chunks"]
    out = np.zeros((N_TOK, C), np.float32)
    for core in range(B):
        yt = res.results[core]["yt"]
        yo = 0
        for e in order:
            capc = capcs[e]
            idx = chunks[e][core]
            n = len(idx)
            if n:
                y = (
                    yt[:, yo:yo + 6 * capc]
                    .reshape(128, 6, capc)[:, :, :n]
                    .transpose(2, 1, 0)
                    .reshape(n, C)
                )
                toks = glists[e][idx]
                out[toks] += wlists[e][idx][:, None] * y
            yo += 6 * capc
    b2 = np.asarray(b2, np.float32)
    if np.any(b2):
        for e in range(E):
            out[glists[e]] += wlists[e][:, None] * b2[e]
    return out.reshape(B, T, C)


def kernel(**inputs):
    _install_ntff_hook()
    from concourse import bass_utils

    in_maps, meta = host_prep(
        inputs["x"], inputs["router_w"], inputs["w1"],
        inputs["b1"], inputs["w2"], inputs["b2"],
    )
    key = ("nc", meta["capcs"], meta["order"])
    if key not in _CACHE:
        _CACHE[key] = build_program(capcs=meta["capcs"], order=meta["order"])
    nc = _CACHE[key]

    res = bass_utils.run_bass_kernel_spmd(
        nc, in_maps, core_ids=list(range(B)), trace=False
    )
    _CACHE["nc"] = nc
    _CACHE["meta"] = meta
    return assemble(res, meta, inputs["b2"])
